# revision 31
# baseline (speedup 1.0000x reference)
"""Distributed Trainium2 kernel for in-batch-negative InfoNCE loss.

loss = mean_i( logsumexp_j( cos(q_i, p_j)/T ) - cos(q_i, p_i)/T )

Strategy (8 NeuronCores, data-parallel over N):
  - each core owns a 1024-row shard of q and p (N=8192, D=768)
  - normalize local p shard (x16), transpose into the fp8 DoubleRow
    interleaved layout [d/256, 2, m], AllGather it in two column-chunks
    (the first collective of a NEFF pays a ~60-70us entry barrier;
    chunking lets h=0 wave compute overlap the second chunk)
  - each core computes its 1024 x 8192 slab of logits with fp8e4m3
    DoubleRow matmuls (157 TF/s), fusing exp + row-sum into a 1024-wide
    ScalarE epilogue per psum pair (logits never touch HBM)
  - q is scaled x8 and NOT normalized before the matmul: the
    1/(128*T*||q_i||) row scale is folded into the exp activation
  - diagonal terms computed separately as an elementwise dot of the
    local q/pn shards in f32/bf16 precision
  - a warm-up wave over the local block runs while the AllGather is in
    flight (sums discarded; keeps PE busy and the HAM clock warm)
  - per-core partial sums are returned per core; host adds 8 scalars
"""

import numpy as np

P = 128          # SBUF partitions
D = 768          # embedding dim
K2 = 3           # 256-deep DoubleRow k-tiles
NL = 1024        # local rows per core
MT = NL // P     # 8 m-tiles per core
NCORES = 8
N = NL * NCORES  # 8192
HALF = 512       # column half per rank block
NSLOT = 10       # rs slots per m-tile: 8 real (h*4+rp) + 1 local + pad
QS = 8.0         # q fp8 pre-scale
PS = 16.0        # pn fp8 pre-scale

_CACHE = {}


def _build(inv_temp: float, n_waves: int = NCORES):
    from concourse import bass, bacc, tile, mybir, masks

    f32 = mybir.dt.float32
    bf16 = mybir.dt.bfloat16
    fp8 = mybir.dt.float8e4
    AF = mybir.ActivationFunctionType
    ALU = mybir.AluOpType
    DR = mybir.MatmulPerfMode.DoubleRow

    nc = bacc.Bacc(
        "TRN2",
        debug=False,
        target_bir_lowering=False,
        num_devices=NCORES,
    )

    q_ext = nc.dram_tensor("q", [NL, D], f32, kind="ExternalInput")
    p_ext = nc.dram_tensor("p", [NL, D], f32, kind="ExternalInput")
    out_ext = nc.dram_tensor("out", [1, 1], f32, kind="ExternalOutput")

    with tile.TileContext(nc) as tc:
        with (
            tc.tile_pool(name="singles", bufs=1) as singles,
            tc.tile_pool(name="persist", bufs=1) as persist,
            tc.tile_pool(name="scr", bufs=4) as scr,
            tc.tile_pool(name="escr", bufs=3) as escr,
            tc.tile_pool(name="stats", bufs=1) as stats,
            tc.tile_pool(name="dram", bufs=1, space="DRAM") as dram,
            tc.tile_pool(name="ps", bufs=2, space="PSUM") as ps,
        ):
            ident_f = singles.tile([P, P], f32)
            masks.make_identity(nc, ident_f[:])
            ident_b = singles.tile([P, P], bf16)
            masks.make_identity(nc, ident_b[:])
            ones = singles.tile([P, 8], f32)
            nc.vector.memset(ones[:], 1.0)
            warm = singles.tile([P, 1], f32)
            nc.vector.memset(warm[:], 0.0)
            bias_p = singles.tile([P, 1], f32)
            nc.vector.memset(bias_p[:], float(np.log(PS)))
            bias_q = singles.tile([P, 1], f32)
            nc.vector.memset(
                bias_q[:], float(np.log(float(inv_temp) / float(QS * PS)))
            )

            # persistent SBUF tensors (fp8 DoubleRow interleaved layouts)
            qT8 = persist.tile([P, K2, 2, NL], fp8)       # 8*q^T
            p_sb = persist.tile([P, MT, D], f32)          # raw p shard
            q_sb = persist.tile([P, MT, D], f32)          # raw q shard
            pT_loc = persist.tile([P, K2, 2, NL], fp8)    # local 16*pn^T
            pn_sb = persist.tile([P, MT, D], bf16)        # local 16*pn
            pT_a = persist.tile([P, NCORES * K2, 2, HALF], fp8)  # cols 0:512
            pT_b = persist.tile([P, NCORES * K2, 2, HALF], fp8)  # cols 512:1024

            # stats
            ssq_p = stats.tile([P, MT], f32)
            ssq_q = stats.tile([P, MT], f32)
            nrm = stats.tile([P, 2 * MT], f32)
            rp = stats.tile([P, MT], f32)        # 16/||p||
            rscale = stats.tile([P, MT], f32)    # 1/(128*T*||q||)
            draw = stats.tile([P, MT], f32)      # raw q.pn16 dots
            dscaled = stats.tile([P, MT], f32)   # diag logits
            rs = stats.tile([P, MT * NSLOT], f32)
            lse_pre = stats.tile([P, MT], f32)
            lse = stats.tile([P, MT], f32)
            vrow = stats.tile([P, MT], f32)
            v1 = stats.tile([P, 1], f32)
            ar_sb = stats.tile([1, 8], f32)

            # DRAM bounce buffers for the collectives
            ag_in_a = dram.tile([D, HALF], fp8)
            ag_in_b = dram.tile([D, HALF], fp8)
            ag_out_a = dram.tile([NCORES * D, HALF], fp8, addr_space="Shared")
            ag_out_b = dram.tile([NCORES * D, HALF], fp8, addr_space="Shared")

            # warm the exp table early
            nc.scalar.activation(warm[:], warm[:], AF.Exp)
            nc.vector.memset(rs[:], 1.0)

            def p_transposes(t):
                pv = pn_sb[:, t, :].rearrange(
                    "p (k2 c two) -> p k2 two c", k2=K2, two=2
                )
                for k2 in range(K2):
                    for s in range(2):
                        tp = ps.tile([P, P], bf16, tag="ps", name="tp")
                        nc.tensor.transpose(tp[:], pv[:, k2, s, :], ident_b[:])
                        nc.vector.tensor_copy(
                            pT_loc[:, k2, s, t * P:(t + 1) * P], tp[:]
                        )

            def wave(t, rhs_fns, slot):
                """len(rhs_fns)*K2 DoubleRow matmuls into a [128, 512*n]
                psum tile (same lhsT reused across the n rhs slices per k2),
                then one fused exp+rowsum. rhs_fns[j]: k2 -> [128,2,512]."""
                n = len(rhs_fns)
                pm = ps.tile([P, n * HALF], f32, tag="ps", name="pm")
                for k2 in range(K2):
                    lhsT = qT8[:, k2, :, t * P:(t + 1) * P]
                    for j, rhs_fn in enumerate(rhs_fns):
                        nc.tensor.matmul(
                            pm[:, j * HALF:(j + 1) * HALF], lhsT, rhs_fn(k2),
                            start=(k2 == 0), stop=(k2 == K2 - 1), perf_mode=DR,
                        )
                ex = escr.tile([P, n * HALF], bf16, tag="ex", name="ex")
                nc.scalar.activation(
                    ex[:], pm[:], AF.Exp,
                    scale=rscale[:, t:t + 1],
                    accum_out=rs[:, slot:slot + 1],
                )

            # ---- loads ---------------------------------------------------
            for t in range(MT):
                nc.sync.dma_start(
                    p_sb[:, t, :], p_ext.ap()[t * P:(t + 1) * P, :]
                )
            for t in range(MT):
                nc.sync.dma_start(
                    q_sb[:, t, :], q_ext.ap()[t * P:(t + 1) * P, :]
                )

            # ---- p norms (batched: one Ln, one Exp — single table set) ---
            for t in range(MT):
                sq = scr.tile([P, D], f32, name="sq")
                nc.vector.scalar_tensor_tensor(
                    out=sq[:], in0=p_sb[:, t, :], scalar=1.0,
                    in1=p_sb[:, t, :], op0=ALU.mult, op1=ALU.mult,
                    accum_out=ssq_p[:, t:t + 1],
                )
            nc.scalar.activation(nrm[:, 0:MT], ssq_p[:, :], AF.Ln)
            # rp = PS/||p|| = exp(-0.5*ln(ssq) + ln(PS))
            nc.scalar.activation(
                rp[:, :], nrm[:, 0:MT], AF.Exp, scale=-0.5, bias=bias_p[:]
            )

            # ---- pn + transposes, AllGathers -----------------------------
            for t in range(MT // 2):
                nc.vector.tensor_scalar_mul(
                    pn_sb[:, t, :], p_sb[:, t, :], rp[:, t:t + 1]
                )
                p_transposes(t)
            nc.sync.dma_start(
                ag_in_a[:].rearrange("(k2 two p) m -> p k2 two m", p=P, two=2),
                pT_loc[:, :, :, 0:HALF],
            )
            nc.gpsimd.collective_compute(
                "AllGather", ALU.bypass,
                replica_groups=[list(range(NCORES))],
                ins=[ag_in_a[:].opt()], outs=[ag_out_a[:].opt()],
            )
            for t in range(MT // 2, MT):
                nc.vector.tensor_scalar_mul(
                    pn_sb[:, t, :], p_sb[:, t, :], rp[:, t:t + 1]
                )
                p_transposes(t)
            nc.sync.dma_start(
                ag_in_b[:].rearrange("(k2 two p) m -> p k2 two m", p=P, two=2),
                pT_loc[:, :, :, HALF:NL],
            )
            nc.gpsimd.collective_compute(
                "AllGather", ALU.bypass,
                replica_groups=[list(range(NCORES))],
                ins=[ag_in_b[:].opt()], outs=[ag_out_b[:].opt()],
            )

            # ---- q norms (batched), diag, q transposes -------------------
            for t in range(MT):
                sq2 = scr.tile([P, D], f32, name="sq2")
                nc.vector.scalar_tensor_tensor(
                    out=sq2[:], in0=q_sb[:, t, :], scalar=1.0,
                    in1=q_sb[:, t, :], op0=ALU.mult, op1=ALU.mult,
                    accum_out=ssq_q[:, t:t + 1],
                )
            nc.scalar.activation(nrm[:, MT:2 * MT], ssq_q[:, :], AF.Ln)
            # rscale = inv_temp/(QS*PS*||q||)
            nc.scalar.activation(
                rscale[:, :], nrm[:, MT:2 * MT], AF.Exp,
                scale=-0.5, bias=bias_q[:],
            )
            for t in range(MT):
                qv = q_sb[:, t, :].rearrange(
                    "p (k2 c two) -> p k2 two c", k2=K2, two=2
                )
                for k2 in range(K2):
                    for s in range(2):
                        tq = ps.tile([P, P], f32, tag="ps", name="tq")
                        nc.tensor.transpose(tq[:], qv[:, k2, s, :], ident_f[:])
                        nc.vector.tensor_scalar_mul(
                            qT8[:, k2, s, t * P:(t + 1) * P], tq[:], float(QS)
                        )
            for t in range(MT):
                dsc = scr.tile([P, D], f32, name="dsc")
                nc.vector.scalar_tensor_tensor(
                    out=dsc[:], in0=q_sb[:, t, :], scalar=1.0,
                    in1=pn_sb[:, t, :], op0=ALU.mult, op1=ALU.mult,
                    accum_out=draw[:, t:t + 1],
                )
            # diag = (q . pn16) * rscale * QS  (rscale carries 1/(QS*PS*T))
            nc.vector.tensor_mul(dscaled[:, :], draw[:, :], rscale[:, :])
            nc.vector.tensor_scalar_mul(dscaled[:, :], dscaled[:, :], float(QS))

            # ---- local warm-up wave (results discarded) ------------------
            for t in range(MT):
                wave(
                    t,
                    [lambda k2: pT_loc[:, k2, :, 0:HALF],
                     lambda k2: pT_loc[:, k2, :, HALF:2 * HALF]],
                    t * NSLOT + 8,
                )

            # ---- stream gathered blocks into SBUF (gpsimd: idle engine,
            # keeps the long AG waits off the compute streams) -------------
            for r in range(NCORES):
                eng = nc.gpsimd if r % 2 else nc.sync
                eng.dma_start(
                    pT_a[:, r * K2:(r + 1) * K2, :, :],
                    ag_out_a[:][r * D:(r + 1) * D, :].rearrange(
                        "(k2 two p) m -> p k2 two m", p=P, two=2
                    ),
                )
            for r in range(NCORES):
                eng = nc.gpsimd if r % 2 else nc.sync
                eng.dma_start(
                    pT_b[:, r * K2:(r + 1) * K2, :, :],
                    ag_out_b[:][r * D:(r + 1) * D, :].rearrange(
                        "(k2 two p) m -> p k2 two m", p=P, two=2
                    ),
                )

            # ---- main waves: h outer, rank-quads inner -------------------
            for h, pT_h in ((0, pT_a), (1, pT_b)):
                for rq in range(n_waves // 4):
                    for t in range(MT):
                        wave(
                            t,
                            [
                                (lambda k2, r=4 * rq + j, pT_h=pT_h:
                                 pT_h[:, r * K2 + k2, :, :])
                                for j in range(4)
                            ],
                            t * NSLOT + h * 2 + rq,
                        )

            # ---- epilogue ------------------------------------------------
            for t in range(MT):
                nc.vector.tensor_reduce(
                    lse_pre[:, t:t + 1],
                    rs[:, t * NSLOT:t * NSLOT + 4],
                    mybir.AxisListType.X,
                    ALU.add,
                )
            nc.scalar.activation(lse[:, :], lse_pre[:, :], AF.Ln)
            nc.vector.tensor_sub(vrow[:, :], lse[:, :], dscaled[:, :])
            nc.vector.tensor_reduce(
                v1[:], vrow[:, :], mybir.AxisListType.X, ALU.add
            )
            pssum = ps.tile([1, 8], f32, tag="ps", name="pssum")
            nc.tensor.matmul(pssum[:], v1[:], ones[:])
            nc.vector.tensor_scalar_mul(ar_sb[:], pssum[:], 1.0 / N)
            nc.sync.dma_start(out_ext.ap(), ar_sb[:, 0:1])

    nc.compile()
    return nc


def _get_nc(inv_temp: float):
    import os
    n_waves = int(os.environ.get("NCE_N_WAVES", NCORES))
    key = (round(float(inv_temp), 9), n_waves)
    if key not in _CACHE:
        _CACHE[key] = _build(inv_temp, n_waves)
    return _CACHE[key]


def kernel(q_emb, p_emb, temperature):
    from concourse.bass_utils import run_bass_kernel_spmd

    q = np.ascontiguousarray(np.asarray(q_emb, dtype=np.float32))
    p = np.ascontiguousarray(np.asarray(p_emb, dtype=np.float32))
    t = float(np.asarray(temperature))
    nc = _get_nc(1.0 / t)

    in_maps = [
        {
            "q": q[i * NL:(i + 1) * NL],
            "p": p[i * NL:(i + 1) * NL],
        }
        for i in range(NCORES)
    ]
    res = run_bass_kernel_spmd(nc, in_maps, core_ids=list(range(NCORES)))
    return np.float32(sum(float(r["out"][0, 0]) for r in res.results))


# revision 39
# speedup vs baseline: 1.0542x; 1.0542x over previous
"""Distributed Trainium2 kernel for in-batch-negative InfoNCE loss.

loss = mean_i( logsumexp_j( cos(q_i, p_j)/T ) - cos(q_i, p_i)/T )

Strategy (8 NeuronCores, data-parallel over N):
  - each core owns a 1024-row shard of q and p (N=8192, D=768)
  - normalize local p shard (x16), transpose into the fp8 DoubleRow
    interleaved layout [d/256, 2, m], AllGather it in two column-chunks
    (the first collective of a NEFF pays a ~60-70us entry barrier;
    chunking lets h=0 wave compute overlap the second chunk)
  - each core computes its 1024 x 8192 slab of logits with fp8e4m3
    DoubleRow matmuls (157 TF/s), fusing exp + row-sum into a 1024-wide
    ScalarE epilogue per psum pair (logits never touch HBM)
  - q is scaled x8 and NOT normalized before the matmul: the
    1/(128*T*||q_i||) row scale is folded into the exp activation
  - diagonal terms computed separately as an elementwise dot of the
    local q/pn shards in f32/bf16 precision
  - a warm-up wave over the local block runs while the AllGather is in
    flight (sums discarded; keeps PE busy and the HAM clock warm)
  - per-core partial sums are returned per core; host adds 8 scalars
"""

import numpy as np

P = 128          # SBUF partitions
D = 768          # embedding dim
K2 = 3           # 256-deep DoubleRow k-tiles
NL = 1024        # local rows per core
MT = NL // P     # 8 m-tiles per core
NCORES = 8
N = NL * NCORES  # 8192
HALF = 512       # column half per rank block
NSLOT = 10       # rs slots per m-tile: 8 real (h*4+rp) + 1 local + pad
QS = 8.0         # q fp8 pre-scale
PS = 16.0        # pn fp8 pre-scale

_CACHE = {}


def _build(inv_temp: float, n_waves: int = NCORES):
    from concourse import bass, bacc, tile, mybir, masks

    f32 = mybir.dt.float32
    bf16 = mybir.dt.bfloat16
    fp8 = mybir.dt.float8e4
    AF = mybir.ActivationFunctionType
    ALU = mybir.AluOpType
    DR = mybir.MatmulPerfMode.DoubleRow

    nc = bacc.Bacc(
        "TRN2",
        debug=False,
        target_bir_lowering=False,
        num_devices=NCORES,
    )

    q_ext = nc.dram_tensor("q", [NL, D], f32, kind="ExternalInput")
    p_ext = nc.dram_tensor("p", [NL, D], f32, kind="ExternalInput")
    out_ext = nc.dram_tensor("out", [1, 1], f32, kind="ExternalOutput")

    with tile.TileContext(nc) as tc:
        with (
            tc.tile_pool(name="singles", bufs=1) as singles,
            tc.tile_pool(name="persist", bufs=1) as persist,
            tc.tile_pool(name="scr", bufs=4) as scr,
            tc.tile_pool(name="escr", bufs=4) as escr,
            tc.tile_pool(name="stats", bufs=1) as stats,
            tc.tile_pool(name="dram", bufs=1, space="DRAM") as dram,
            tc.tile_pool(name="ps", bufs=2, space="PSUM") as ps,
        ):
            ident_f = singles.tile([P, P], f32)
            masks.make_identity(nc, ident_f[:])
            ident_b = singles.tile([P, P], bf16)
            masks.make_identity(nc, ident_b[:])
            ones = singles.tile([P, 8], f32)
            nc.vector.memset(ones[:], 1.0)
            warm = singles.tile([P, 1], f32)
            nc.vector.memset(warm[:], 0.0)
            bias_p = singles.tile([P, 1], f32)
            nc.vector.memset(bias_p[:], float(np.log(PS)))
            bias_q = singles.tile([P, 1], f32)
            nc.vector.memset(
                bias_q[:], float(np.log(float(inv_temp) / float(QS * PS)))
            )

            # persistent SBUF tensors (fp8 DoubleRow interleaved layouts)
            qT8 = persist.tile([P, K2, 2, NL], fp8)       # 8*q^T
            p_sb = persist.tile([P, MT, D], f32)          # raw p shard
            q_sb = persist.tile([P, MT, D], f32)          # raw q shard
            pT_loc = persist.tile([P, K2, 2, NL], fp8)    # local 16*pn^T
            pn_sb = persist.tile([P, MT, D], bf16)        # local 16*pn
            pT_a = persist.tile([P, NCORES * K2, 2, HALF], fp8)  # cols 0:512
            pT_b = persist.tile([P, NCORES * K2, 2, HALF], fp8)  # cols 512:1024

            # stats
            ssq_p = stats.tile([P, MT], f32)
            ssq_q = stats.tile([P, MT], f32)
            nrm = stats.tile([P, 2 * MT], f32)
            rp = stats.tile([P, MT], f32)        # 16/||p||
            rscale = stats.tile([P, MT], f32)    # 1/(128*T*||q||)
            draw = stats.tile([P, MT], f32)      # raw q.pn16 dots
            dscaled = stats.tile([P, MT], f32)   # diag logits
            rs = stats.tile([P, MT * NSLOT], f32)
            lse_pre = stats.tile([P, MT], f32)
            lse = stats.tile([P, MT], f32)
            vrow = stats.tile([P, MT], f32)
            v1 = stats.tile([P, 1], f32)
            ar_sb = stats.tile([1, 8], f32)

            # DRAM bounce buffers for the collectives
            ag_in_a = dram.tile([D, HALF], fp8)
            ag_in_b = dram.tile([D, HALF], fp8)
            ag_out_a = dram.tile([NCORES * D, HALF], fp8, addr_space="Shared")
            ag_out_b = dram.tile([NCORES * D, HALF], fp8, addr_space="Shared")

            # warm the exp table early
            nc.scalar.activation(warm[:], warm[:], AF.Exp)
            nc.vector.memset(rs[:], 1.0)

            def p_transposes(t):
                pv = pn_sb[:, t, :].rearrange(
                    "p (k2 c two) -> p k2 two c", k2=K2, two=2
                )
                for k2 in range(K2):
                    for s in range(2):
                        tp = ps.tile([P, P], bf16, tag="ps", name="tp")
                        nc.tensor.transpose(tp[:], pv[:, k2, s, :], ident_b[:])
                        nc.vector.tensor_copy(
                            pT_loc[:, k2, s, t * P:(t + 1) * P], tp[:]
                        )

            def wave(t, rhs_fns, slot):
                """len(rhs_fns)*K2 DoubleRow matmuls into a [128, 512*n]
                psum tile (same lhsT reused across the n rhs slices per k2),
                then one fused exp+rowsum. rhs_fns[j]: k2 -> [128,2,512]."""
                n = len(rhs_fns)
                pm = ps.tile([P, n * HALF], f32, tag="ps", name="pm")
                for k2 in range(K2):
                    lhsT = qT8[:, k2, :, t * P:(t + 1) * P]
                    for j, rhs_fn in enumerate(rhs_fns):
                        nc.tensor.matmul(
                            pm[:, j * HALF:(j + 1) * HALF], lhsT, rhs_fn(k2),
                            start=(k2 == 0), stop=(k2 == K2 - 1), perf_mode=DR,
                        )
                ex = escr.tile([P, n * HALF], bf16, tag="ex", name="ex")
                nc.scalar.activation(
                    ex[:], pm[:], AF.Exp,
                    scale=rscale[:, t:t + 1],
                    accum_out=rs[:, slot:slot + 1],
                )

            # ---- loads ---------------------------------------------------
            for t in range(MT):
                nc.sync.dma_start(
                    p_sb[:, t, :], p_ext.ap()[t * P:(t + 1) * P, :]
                )
            for t in range(MT):
                nc.sync.dma_start(
                    q_sb[:, t, :], q_ext.ap()[t * P:(t + 1) * P, :]
                )

            # ---- p norms (batched by half: Ln/Exp share one table set) ---
            H4 = MT // 2
            for t in range(H4):
                sq = scr.tile([P, D], f32, name="sq")
                nc.vector.scalar_tensor_tensor(
                    out=sq[:], in0=p_sb[:, t, :], scalar=1.0,
                    in1=p_sb[:, t, :], op0=ALU.mult, op1=ALU.mult,
                    accum_out=ssq_p[:, t:t + 1],
                )
            nc.scalar.activation(nrm[:, 0:H4], ssq_p[:, 0:H4], AF.Ln)
            # rp = PS/||p|| = exp(-0.5*ln(ssq) + ln(PS))
            nc.scalar.activation(
                rp[:, 0:H4], nrm[:, 0:H4], AF.Exp, scale=-0.5, bias=bias_p[:]
            )
            for t in range(H4, MT):
                sq = scr.tile([P, D], f32, name="sq")
                nc.vector.scalar_tensor_tensor(
                    out=sq[:], in0=p_sb[:, t, :], scalar=1.0,
                    in1=p_sb[:, t, :], op0=ALU.mult, op1=ALU.mult,
                    accum_out=ssq_p[:, t:t + 1],
                )
            nc.scalar.activation(nrm[:, H4:MT], ssq_p[:, H4:MT], AF.Ln)
            nc.scalar.activation(
                rp[:, H4:MT], nrm[:, H4:MT], AF.Exp, scale=-0.5, bias=bias_p[:]
            )

            # ---- pn + transposes, AllGathers -----------------------------
            for t in range(MT // 2):
                nc.vector.tensor_scalar_mul(
                    pn_sb[:, t, :], p_sb[:, t, :], rp[:, t:t + 1]
                )
                p_transposes(t)
            nc.sync.dma_start(
                ag_in_a[:].rearrange("(k2 two p) m -> p k2 two m", p=P, two=2),
                pT_loc[:, :, :, 0:HALF],
            )
            nc.gpsimd.collective_compute(
                "AllGather", ALU.bypass,
                replica_groups=[list(range(NCORES))],
                ins=[ag_in_a[:].opt()], outs=[ag_out_a[:].opt()],
            )
            for t in range(MT // 2, MT):
                nc.vector.tensor_scalar_mul(
                    pn_sb[:, t, :], p_sb[:, t, :], rp[:, t:t + 1]
                )
                p_transposes(t)
            nc.sync.dma_start(
                ag_in_b[:].rearrange("(k2 two p) m -> p k2 two m", p=P, two=2),
                pT_loc[:, :, :, HALF:NL],
            )
            nc.gpsimd.collective_compute(
                "AllGather", ALU.bypass,
                replica_groups=[list(range(NCORES))],
                ins=[ag_in_b[:].opt()], outs=[ag_out_b[:].opt()],
            )

            # ---- q norms (batched), diag, q transposes -------------------
            for t in range(MT):
                sq2 = scr.tile([P, D], f32, name="sq2")
                nc.vector.scalar_tensor_tensor(
                    out=sq2[:], in0=q_sb[:, t, :], scalar=1.0,
                    in1=q_sb[:, t, :], op0=ALU.mult, op1=ALU.mult,
                    accum_out=ssq_q[:, t:t + 1],
                )
            nc.scalar.activation(nrm[:, MT:2 * MT], ssq_q[:, :], AF.Ln)
            # rscale = inv_temp/(QS*PS*||q||)
            nc.scalar.activation(
                rscale[:, :], nrm[:, MT:2 * MT], AF.Exp,
                scale=-0.5, bias=bias_q[:],
            )
            for t in range(MT):
                qv = q_sb[:, t, :].rearrange(
                    "p (k2 c two) -> p k2 two c", k2=K2, two=2
                )
                for k2 in range(K2):
                    for s in range(2):
                        tq = ps.tile([P, P], f32, tag="ps", name="tq")
                        nc.tensor.transpose(tq[:], qv[:, k2, s, :], ident_f[:])
                        nc.vector.tensor_scalar_mul(
                            qT8[:, k2, s, t * P:(t + 1) * P], tq[:], float(QS)
                        )
                # local wave immediately after this row-tile's transposes:
                # fills the pre-AllGather window with counted work
                wave(
                    t,
                    [lambda k2: pT_loc[:, k2, :, 0:HALF],
                     lambda k2: pT_loc[:, k2, :, HALF:2 * HALF]],
                    t * NSLOT + 4,
                )
            for t in range(MT):
                dsc = scr.tile([P, D], f32, name="dsc")
                nc.vector.scalar_tensor_tensor(
                    out=dsc[:], in0=q_sb[:, t, :], scalar=1.0,
                    in1=pn_sb[:, t, :], op0=ALU.mult, op1=ALU.mult,
                    accum_out=draw[:, t:t + 1],
                )
            # diag = (q . pn16) * rscale * QS  (rscale carries 1/(QS*PS*T))
            nc.vector.tensor_mul(dscaled[:, :], draw[:, :], rscale[:, :])
            nc.vector.tensor_scalar_mul(dscaled[:, :], dscaled[:, :], float(QS))


            # ---- stream gathered blocks into SBUF (gpsimd: idle engine,
            # keeps the long AG waits off the compute streams) -------------
            # rotated: slot j holds block (me+1+j)%8 — skips the local
            # block (covered by the counted warm-up wave from pT_loc)
            me_sync = nc.sync.partition_id()
            me_gp = nc.gpsimd.partition_id()
            for j in range(NCORES - 1):
                eng, me = ((nc.sync, me_sync), (nc.gpsimd, me_gp))[j % 2]
                roff = ((me + (1 + j)) % NCORES) * D
                eng.dma_start(
                    pT_a[:, j * K2:(j + 1) * K2, :, :],
                    ag_out_a[:][bass.ds(roff, D), :].rearrange(
                        "(k2 two p) m -> p k2 two m", p=P, two=2
                    ),
                )
            for j in range(NCORES - 1):
                eng, me = ((nc.gpsimd, me_gp), (nc.sync, me_sync))[j % 2]
                roff = ((me + (1 + j)) % NCORES) * D
                eng.dma_start(
                    pT_b[:, j * K2:(j + 1) * K2, :, :],
                    ag_out_b[:][bass.ds(roff, D), :].rearrange(
                        "(k2 two p) m -> p k2 two m", p=P, two=2
                    ),
                )

            # ---- main waves: h0 quad-outer; h1 t-outer so each row
            # tile's epilogue can drain while later tiles still compute ----
            for rq, slots in enumerate(([0, 1, 2, 3], [4, 5, 6])):
                for t in range(MT):
                    wave(
                        t,
                        [(lambda k2, r=r: pT_a[:, r * K2 + k2, :, :])
                         for r in slots],
                        t * NSLOT + rq,
                    )
            for t in range(MT):
                for rq, slots in enumerate(([0, 1, 2, 3], [4, 5, 6])):
                    wave(
                        t,
                        [(lambda k2, r=r: pT_b[:, r * K2 + k2, :, :])
                         for r in slots],
                        t * NSLOT + 2 + rq,
                    )
                nc.vector.tensor_reduce(
                    lse_pre[:, t:t + 1],
                    rs[:, t * NSLOT:t * NSLOT + 5],
                    mybir.AxisListType.X,
                    ALU.add,
                )

            # ---- epilogue ------------------------------------------------
            nc.scalar.activation(lse[:, :], lse_pre[:, :], AF.Ln)
            nc.vector.tensor_sub(vrow[:, :], lse[:, :], dscaled[:, :])
            nc.vector.tensor_reduce(
                v1[:], vrow[:, :], mybir.AxisListType.X, ALU.add
            )
            pssum = ps.tile([1, 8], f32, tag="ps", name="pssum")
            nc.tensor.matmul(pssum[:], v1[:], ones[:])
            nc.vector.tensor_scalar_mul(ar_sb[:], pssum[:], 1.0 / N)
            nc.sync.dma_start(out_ext.ap(), ar_sb[:, 0:1])

    nc.compile()
    return nc


def _get_nc(inv_temp: float):
    import os
    n_waves = int(os.environ.get("NCE_N_WAVES", NCORES))
    key = (round(float(inv_temp), 9), n_waves)
    if key not in _CACHE:
        _CACHE[key] = _build(inv_temp, n_waves)
    return _CACHE[key]


def kernel(q_emb, p_emb, temperature):
    from concourse.bass_utils import run_bass_kernel_spmd

    q = np.ascontiguousarray(np.asarray(q_emb, dtype=np.float32))
    p = np.ascontiguousarray(np.asarray(p_emb, dtype=np.float32))
    t = float(np.asarray(temperature))
    nc = _get_nc(1.0 / t)

    in_maps = [
        {
            "q": q[i * NL:(i + 1) * NL],
            "p": p[i * NL:(i + 1) * NL],
        }
        for i in range(NCORES)
    ]
    res = run_bass_kernel_spmd(nc, in_maps, core_ids=list(range(NCORES)))
    return np.float32(sum(float(r["out"][0, 0]) for r in res.results))


# revision 40
# speedup vs baseline: 1.0586x; 1.0042x over previous
"""Distributed Trainium2 kernel for in-batch-negative InfoNCE loss.

loss = mean_i( logsumexp_j( cos(q_i, p_j)/T ) - cos(q_i, p_i)/T )

Strategy (8 NeuronCores, data-parallel over N):
  - each core owns a 1024-row shard of q and p (N=8192, D=768)
  - normalize local p shard (x16), transpose into the fp8 DoubleRow
    interleaved layout [d/256, 2, m], AllGather it in two column-chunks
    (the first collective of a NEFF pays a ~60-70us entry barrier;
    chunking lets h=0 wave compute overlap the second chunk)
  - each core computes its 1024 x 8192 slab of logits with fp8e4m3
    DoubleRow matmuls (157 TF/s), fusing exp + row-sum into a 1024-wide
    ScalarE epilogue per psum pair (logits never touch HBM)
  - q is scaled x8 and NOT normalized before the matmul: the
    1/(128*T*||q_i||) row scale is folded into the exp activation
  - diagonal terms computed separately as an elementwise dot of the
    local q/pn shards in f32/bf16 precision
  - a warm-up wave over the local block runs while the AllGather is in
    flight (sums discarded; keeps PE busy and the HAM clock warm)
  - per-core partial sums are returned per core; host adds 8 scalars
"""

import numpy as np

P = 128          # SBUF partitions
D = 768          # embedding dim
K2 = 3           # 256-deep DoubleRow k-tiles
NL = 1024        # local rows per core
MT = NL // P     # 8 m-tiles per core
NCORES = 8
N = NL * NCORES  # 8192
HALF = 512       # column half per rank block
NSLOT = 10       # rs slots per m-tile: 8 real (h*4+rp) + 1 local + pad
QS = 8.0         # q fp8 pre-scale
PS = 16.0        # pn fp8 pre-scale

_CACHE = {}


def _build(inv_temp: float, n_waves: int = NCORES):
    from concourse import bass, bacc, tile, mybir, masks

    f32 = mybir.dt.float32
    bf16 = mybir.dt.bfloat16
    fp8 = mybir.dt.float8e4
    AF = mybir.ActivationFunctionType
    ALU = mybir.AluOpType
    DR = mybir.MatmulPerfMode.DoubleRow

    nc = bacc.Bacc(
        "TRN2",
        debug=False,
        target_bir_lowering=False,
        num_devices=NCORES,
    )

    q_ext = nc.dram_tensor("q", [NL, D], f32, kind="ExternalInput")
    p_ext = nc.dram_tensor("p", [NL, D], f32, kind="ExternalInput")
    out_ext = nc.dram_tensor("out", [1, 1], f32, kind="ExternalOutput")

    with tile.TileContext(nc) as tc:
        with (
            tc.tile_pool(name="singles", bufs=1) as singles,
            tc.tile_pool(name="persist", bufs=1) as persist,
            tc.tile_pool(name="scr", bufs=4) as scr,
            tc.tile_pool(name="escr", bufs=4) as escr,
            tc.tile_pool(name="stats", bufs=1) as stats,
            tc.tile_pool(name="dram", bufs=1, space="DRAM") as dram,
            tc.tile_pool(name="ps", bufs=2, space="PSUM") as ps,
        ):
            ident_f = singles.tile([P, P], f32)
            masks.make_identity(nc, ident_f[:])
            ident_b = singles.tile([P, P], bf16)
            masks.make_identity(nc, ident_b[:])
            ones = singles.tile([P, 8], f32)
            nc.vector.memset(ones[:], 1.0)
            warm = singles.tile([P, 1], f32)
            nc.vector.memset(warm[:], 0.0)
            bias_p = singles.tile([P, 1], f32)
            nc.vector.memset(bias_p[:], float(np.log(PS)))
            bias_q = singles.tile([P, 1], f32)
            nc.vector.memset(
                bias_q[:], float(np.log(float(inv_temp) / float(QS * PS)))
            )

            # persistent SBUF tensors (fp8 DoubleRow interleaved layouts)
            qT8 = persist.tile([P, K2, 2, NL], fp8)       # 8*q^T
            p_sb = persist.tile([P, MT, D], f32)          # raw p shard
            q_sb = persist.tile([P, MT, D], f32)          # raw q shard
            pT_loc = persist.tile([P, K2, 2, NL], fp8)    # local 16*pn^T
            pn_sb = persist.tile([P, MT, D], bf16)        # local 16*pn
            pT_a = persist.tile([P, NCORES * K2, 2, HALF], fp8)  # cols 0:512
            pT_b = persist.tile([P, NCORES * K2, 2, HALF], fp8)  # cols 512:1024

            # stats
            ssq_p = stats.tile([P, MT], f32)
            ssq_q = stats.tile([P, MT], f32)
            nrm = stats.tile([P, 2 * MT], f32)
            rp = stats.tile([P, MT], f32)        # 16/||p||
            rscale = stats.tile([P, MT], f32)    # 1/(128*T*||q||)
            draw = stats.tile([P, MT], f32)      # raw q.pn16 dots
            dscaled = stats.tile([P, MT], f32)   # diag logits
            rs = stats.tile([P, MT * NSLOT], f32)
            lse_pre = stats.tile([P, MT], f32)
            lse = stats.tile([P, MT], f32)
            vrow = stats.tile([P, MT], f32)
            v1 = stats.tile([P, 1], f32)
            ar_sb = stats.tile([1, 8], f32)

            # DRAM bounce buffers for the collectives
            ag_in_a = dram.tile([D, HALF], fp8)
            ag_in_b = dram.tile([D, HALF], fp8)
            ag_out_a = dram.tile([NCORES * D, HALF], fp8, addr_space="Shared")
            ag_out_b = dram.tile([NCORES * D, HALF], fp8, addr_space="Shared")

            # warm the exp table early
            nc.scalar.activation(warm[:], warm[:], AF.Exp)
            nc.vector.memset(rs[:], 1.0)

            def p_transposes(t):
                pv = pn_sb[:, t, :].rearrange(
                    "p (k2 c two) -> p k2 two c", k2=K2, two=2
                )
                for k2 in range(K2):
                    for s in range(2):
                        tp = ps.tile([P, P], bf16, tag="ps", name="tp")
                        nc.tensor.transpose(tp[:], pv[:, k2, s, :], ident_b[:])
                        nc.vector.tensor_copy(
                            pT_loc[:, k2, s, t * P:(t + 1) * P], tp[:]
                        )

            def wave(t, rhs_fns, slot):
                """len(rhs_fns)*K2 DoubleRow matmuls into a [128, 512*n]
                psum tile (same lhsT reused across the n rhs slices per k2),
                then one fused exp+rowsum. rhs_fns[j]: k2 -> [128,2,512]."""
                n = len(rhs_fns)
                pm = ps.tile([P, n * HALF], f32, tag="ps", name="pm")
                for k2 in range(K2):
                    lhsT = qT8[:, k2, :, t * P:(t + 1) * P]
                    for j, rhs_fn in enumerate(rhs_fns):
                        nc.tensor.matmul(
                            pm[:, j * HALF:(j + 1) * HALF], lhsT, rhs_fn(k2),
                            start=(k2 == 0), stop=(k2 == K2 - 1), perf_mode=DR,
                        )
                ex = escr.tile([P, n * HALF], bf16, tag="ex", name="ex")
                nc.scalar.activation(
                    ex[:], pm[:], AF.Exp,
                    scale=rscale[:, t:t + 1],
                    accum_out=rs[:, slot:slot + 1],
                )

            # ---- loads ---------------------------------------------------
            for t in range(MT):
                nc.sync.dma_start(
                    p_sb[:, t, :], p_ext.ap()[t * P:(t + 1) * P, :]
                )
            for t in range(MT):
                nc.sync.dma_start(
                    q_sb[:, t, :], q_ext.ap()[t * P:(t + 1) * P, :]
                )

            # ---- p norms (batched by half: Ln/Exp share one table set) ---
            H4 = MT // 2
            for t in range(H4):
                sq = scr.tile([P, D], f32, name="sq")
                nc.vector.scalar_tensor_tensor(
                    out=sq[:], in0=p_sb[:, t, :], scalar=1.0,
                    in1=p_sb[:, t, :], op0=ALU.mult, op1=ALU.mult,
                    accum_out=ssq_p[:, t:t + 1],
                )
            nc.scalar.activation(nrm[:, 0:H4], ssq_p[:, 0:H4], AF.Ln)
            # rp = PS/||p|| = exp(-0.5*ln(ssq) + ln(PS))
            nc.scalar.activation(
                rp[:, 0:H4], nrm[:, 0:H4], AF.Exp, scale=-0.5, bias=bias_p[:]
            )
            for t in range(H4, MT):
                sq = scr.tile([P, D], f32, name="sq")
                nc.vector.scalar_tensor_tensor(
                    out=sq[:], in0=p_sb[:, t, :], scalar=1.0,
                    in1=p_sb[:, t, :], op0=ALU.mult, op1=ALU.mult,
                    accum_out=ssq_p[:, t:t + 1],
                )
            nc.scalar.activation(nrm[:, H4:MT], ssq_p[:, H4:MT], AF.Ln)
            nc.scalar.activation(
                rp[:, H4:MT], nrm[:, H4:MT], AF.Exp, scale=-0.5, bias=bias_p[:]
            )

            # ---- pn + transposes, AllGathers -----------------------------
            for t in range(MT // 2):
                nc.vector.tensor_scalar_mul(
                    pn_sb[:, t, :], p_sb[:, t, :], rp[:, t:t + 1]
                )
                p_transposes(t)
            nc.sync.dma_start(
                ag_in_a[:].rearrange("(k2 two p) m -> p k2 two m", p=P, two=2),
                pT_loc[:, :, :, 0:HALF],
            )
            nc.gpsimd.collective_compute(
                "AllGather", ALU.bypass,
                replica_groups=[list(range(NCORES))],
                ins=[ag_in_a[:].opt()], outs=[ag_out_a[:].opt()],
            )
            for t in range(MT // 2, MT):
                nc.vector.tensor_scalar_mul(
                    pn_sb[:, t, :], p_sb[:, t, :], rp[:, t:t + 1]
                )
                p_transposes(t)
            nc.sync.dma_start(
                ag_in_b[:].rearrange("(k2 two p) m -> p k2 two m", p=P, two=2),
                pT_loc[:, :, :, HALF:NL],
            )
            nc.gpsimd.collective_compute(
                "AllGather", ALU.bypass,
                replica_groups=[list(range(NCORES))],
                ins=[ag_in_b[:].opt()], outs=[ag_out_b[:].opt()],
            )

            # ---- q norms (batched), diag, q transposes -------------------
            for t in range(MT):
                sq2 = scr.tile([P, D], f32, name="sq2")
                nc.vector.scalar_tensor_tensor(
                    out=sq2[:], in0=q_sb[:, t, :], scalar=1.0,
                    in1=q_sb[:, t, :], op0=ALU.mult, op1=ALU.mult,
                    accum_out=ssq_q[:, t:t + 1],
                )
            nc.scalar.activation(nrm[:, MT:2 * MT], ssq_q[:, :], AF.Ln)
            # rscale = inv_temp/(QS*PS*||q||)
            nc.scalar.activation(
                rscale[:, :], nrm[:, MT:2 * MT], AF.Exp,
                scale=-0.5, bias=bias_q[:],
            )
            for t in range(MT):
                qv = q_sb[:, t, :].rearrange(
                    "p (k2 c two) -> p k2 two c", k2=K2, two=2
                )
                for k2 in range(K2):
                    for s in range(2):
                        tq = ps.tile([P, P], f32, tag="ps", name="tq")
                        nc.tensor.transpose(tq[:], qv[:, k2, s, :], ident_f[:])
                        nc.vector.tensor_scalar_mul(
                            qT8[:, k2, s, t * P:(t + 1) * P], tq[:], float(QS)
                        )
                # local wave immediately after this row-tile's transposes:
                # fills the pre-AllGather window with counted work
                wave(
                    t,
                    [lambda k2: pT_loc[:, k2, :, 0:HALF],
                     lambda k2: pT_loc[:, k2, :, HALF:2 * HALF]],
                    t * NSLOT + 4,
                )
            for t in range(MT):
                dsc = scr.tile([P, D], f32, name="dsc")
                nc.vector.scalar_tensor_tensor(
                    out=dsc[:], in0=q_sb[:, t, :], scalar=1.0,
                    in1=pn_sb[:, t, :], op0=ALU.mult, op1=ALU.mult,
                    accum_out=draw[:, t:t + 1],
                )
            # diag = (q . pn16) * rscale * QS  (rscale carries 1/(QS*PS*T))
            nc.vector.tensor_mul(dscaled[:, :], draw[:, :], rscale[:, :])
            nc.vector.tensor_scalar_mul(dscaled[:, :], dscaled[:, :], float(QS))


            # ---- stream gathered blocks into SBUF (gpsimd: idle engine,
            # keeps the long AG waits off the compute streams) -------------
            # rotated: slot j holds block (me+1+j)%8 — skips the local
            # block (covered by the counted warm-up wave from pT_loc)
            me_sync = nc.sync.partition_id()
            me_gp = nc.gpsimd.partition_id()
            for j in range(NCORES - 1):
                eng, me = ((nc.sync, me_sync), (nc.gpsimd, me_gp))[j % 2]
                roff = ((me + (1 + j)) % NCORES) * D
                eng.dma_start(
                    pT_a[:, j * K2:(j + 1) * K2, :, :],
                    ag_out_a[:][bass.ds(roff, D), :].rearrange(
                        "(k2 two p) m -> p k2 two m", p=P, two=2
                    ),
                )
            for j in range(NCORES - 1):
                eng, me = ((nc.gpsimd, me_gp), (nc.sync, me_sync))[j % 2]
                roff = ((me + (1 + j)) % NCORES) * D
                eng.dma_start(
                    pT_b[:, j * K2:(j + 1) * K2, :, :],
                    ag_out_b[:][bass.ds(roff, D), :].rearrange(
                        "(k2 two p) m -> p k2 two m", p=P, two=2
                    ),
                )

            # ---- main waves: h0 quad-outer; h1 t-outer so each row
            # tile's epilogue can drain while later tiles still compute ----
            for rq, slots in enumerate(([0, 1, 2, 3], [4, 5, 6])):
                for t in range(MT):
                    wave(
                        t,
                        [(lambda k2, r=r: pT_a[:, r * K2 + k2, :, :])
                         for r in slots],
                        t * NSLOT + rq,
                    )
            for t in range(MT):
                for rq, slots in enumerate(([0, 1, 2, 3], [4, 5, 6])):
                    wave(
                        t,
                        [(lambda k2, r=r: pT_b[:, r * K2 + k2, :, :])
                         for r in slots],
                        t * NSLOT + 2 + rq,
                    )
                nc.vector.tensor_reduce(
                    lse_pre[:, t:t + 1],
                    rs[:, t * NSLOT:t * NSLOT + 5],
                    mybir.AxisListType.X,
                    ALU.add,
                )

            # ---- epilogue ------------------------------------------------
            nc.scalar.activation(lse[:, :], lse_pre[:, :], AF.Ln)
            nc.vector.tensor_sub(vrow[:, :], lse[:, :], dscaled[:, :])
            nc.vector.tensor_reduce(
                v1[:], vrow[:, :], mybir.AxisListType.X, ALU.add
            )
            pssum = ps.tile([1, 8], f32, tag="ps", name="pssum")
            nc.tensor.matmul(pssum[:], v1[:], ones[:])
            nc.vector.tensor_scalar_mul(ar_sb[:], pssum[:], 1.0 / N)
            nc.sync.dma_start(out_ext.ap(), ar_sb[:, 0:1])

    nc.compile()
    return nc


def _get_nc(inv_temp: float):
    import os
    n_waves = int(os.environ.get("NCE_N_WAVES", NCORES))
    key = (round(float(inv_temp), 9), n_waves)
    if key not in _CACHE:
        _CACHE[key] = _build(inv_temp, n_waves)
    return _CACHE[key]


def kernel(q_emb, p_emb, temperature):
    from concourse.bass_utils import run_bass_kernel_spmd

    q = np.ascontiguousarray(np.asarray(q_emb, dtype=np.float32))
    p = np.ascontiguousarray(np.asarray(p_emb, dtype=np.float32))
    t = float(np.asarray(temperature))
    nc = _get_nc(1.0 / t)

    in_maps = [
        {
            "q": q[i * NL:(i + 1) * NL],
            "p": p[i * NL:(i + 1) * NL],
        }
        for i in range(NCORES)
    ]
    try:
        res = run_bass_kernel_spmd(nc, in_maps, core_ids=list(range(NCORES)))
    except Exception:
        # transient device/tunnel hiccups happen; retry once on a fresh call
        import time
        time.sleep(5)
        res = run_bass_kernel_spmd(nc, in_maps, core_ids=list(range(NCORES)))
    return np.float32(sum(float(r["out"][0, 0]) for r in res.results))
